# revision 23
# baseline (speedup 1.0000x reference)
"""Trainium2 Bass kernel for nn_ActionSmoothingLoss.

Math (per row y of previous_actions, x = segmented log_softmax(current_action)):
    e = exp(y)                       (no max-subtraction: |y| <= ~5.5, safe in f32)
    Z_j = sum_{i in seg j} e_i
    S_j = sum_{i in seg j} e_i * (y_i - x_i)
    loss = (1/W) * sum_rows sum_j inv_n_j * [ S_j / Z_j - log Z_j ]

Sharding: data-parallel over W across 8 cores; x replicated; partial sums
(per-partition accumulators) gathered and combined on host.

Device layout per core (W_core = 65536 rows):
    T=8 tiles of [128 partitions, R=64 rows * 68], rows contiguous per
    partition (contiguous 17KB DMA bursts; loaded as two half-tile DMAs to
    halve pipeline fill).
    ACT: exp (bulk), per-segment log with hardware accumulation (accum_out);
         both functions forced into one activation-table set (no reloads).
    DVE (bottleneck, ~96% busy): subtract(x-broadcast), multiply, 6 merged
         Z/S segment reduces per tile (4D APs covering e and u at once),
         reciprocal_approx_fast, 4 scalar_tensor_tensor ops folding inv_n
         and accumulating S/Z partials into per-(partition, tile, group)
         columns. Final combine (few KB) happens on host in float64.

Measured on 8 axon TRN2 cores: ~179 us HW exec, rel err ~8e-6
(HBM roofline for the 142.6MB stream is ~50us/core; DVE fp32 streaming
is the binding constraint: reduces are capped at 1 elem/cycle/lane and
tensor_tensor at 1 out/cycle/lane, 4 full passes total).
"""

import sys

sys.path.insert(0, "/opt/trn_rl_repo")

import numpy as np

NVEC = (3, 3, 4, 25, 25, 8)
OFFS = (0, 3, 6, 10, 35, 60)
A = 68
P = 128
N_CORES = 8
W_FULL = 524288
W_CORE = W_FULL // N_CORES  # 65536
R = 64                      # rows per partition per tile
F = R * A                   # 4352 free elems per tile
T = W_CORE // (P * R)       # 8 tiles per core

_PROGRAM_CACHE = {}


def build_program(w_core=W_CORE, r=R):
    import concourse.bass as bass
    import concourse.bacc as bacc
    import concourse.mybir as mybir
    from concourse import tile

    f32 = mybir.dt.float32
    Ft = r * A
    Tt = w_core // (P * r)
    assert Tt * P * r == w_core

    Exp = mybir.ActivationFunctionType.Exp
    Ln = mybir.ActivationFunctionType.Ln
    # GpSimd offload measured as a net loss: its SBUF port is shared with
    # DVE's second read port, and concurrent GpSimd streaming inflated every
    # 2-port DVE op (TT +39%, scalar_tensor_tensor 5x). All work stays on DVE.
    # bf16 e/d/u was tried for the 2x multiply mode: systematic ~3.5e-4 bias
    # (not worth it); everything stays fp32.
    sub_op = mybir.AluOpType.subtract
    mult_op = mybir.AluOpType.mult
    add_op = mybir.AluOpType.add
    AX = mybir.AxisListType.X

    nc = bacc.Bacc(None, target_bir_lowering=False)
    pa = nc.dram_tensor("pa", [w_core, A], f32, kind="ExternalInput")
    xb = nc.dram_tensor("xb", [P, A], f32, kind="ExternalInput")
    acc_a = nc.dram_tensor("acc_a", [P, Tt * 4], f32, kind="ExternalOutput")
    acc_b = nc.dram_tensor("acc_b", [P, Tt * 6], f32, kind="ExternalOutput")

    pav = pa.rearrange("(t p r) a -> t p (r a)", t=Tt, p=P, r=r)

    with tile.TileContext(nc) as tc:
        with tc.tile_pool(name="io", bufs=3) as io, \
             tc.tile_pool(name="wk", bufs=2) as wk, \
             tc.tile_pool(name="sm", bufs=2) as sm, \
             tc.tile_pool(name="ps", bufs=1) as ps:
            xbt = ps.tile([P, A], f32)
            nc.sync.dma_start(xbt[:], xb[:])
            accA = ps.tile([P, Tt * 4], f32)
            accB = ps.tile([P, Tt * 6], f32)
            # Materialize x broadcast to [P, r*A] once: single consumer of the
            # xb DMA (keeps per-tile TT instructions at one sem wait each).
            xbb = ps.tile([P, Ft], f32)
            nc.vector.tensor_copy(
                xbb[:].rearrange("p (r a) -> p r a", r=r),
                xbt[:].unsqueeze(1).broadcast_to((P, r, A)))
            # stt groups: adjacent segments with equal n share one
            # scalar_tensor_tensor (same inv_n immediate).
            STT_GROUPS = [(0, 2, 3), (2, 1, 4), (3, 2, 25), (5, 1, 8)]
            H = Ft // 2
            for t in range(Tt):
                y = io.tile([P, Ft], f32, tag="y")
                # Two half-DMAs: halves the pipeline fill time (the first
                # subtract starts after ~1.1MB instead of ~2.2MB).
                nc.sync.dma_start(y[:, :H], pav[t][:, :H])
                nc.sync.dma_start(y[:, H:], pav[t][:, H:])
                # e and u live in one tile so each segment's Z and S come
                # from a single 4D-AP reduce instruction.
                eu = wk.tile([P, 2 * Ft], f32, tag="eu")
                d = wk.tile([P, Ft], f32, tag="d")
                for h in (slice(0, H), slice(H, Ft)):
                    nc.scalar.activation(eu[:, h], y[:, h], Exp)
                    nc.vector.tensor_tensor(d[:, h], y[:, h], xbb[:, h], op=sub_op)
                nc.vector.tensor_tensor(eu[:, Ft:], eu[:, :Ft], d[:], op=mult_op)
                eu4 = eu[:].rearrange("p (k r a) -> p k r a", k=2, r=r)
                ZS = sm.tile([P, 2 * 6 * r], f32, tag="ZS")
                ZS3 = ZS[:].rearrange("p (k q) -> p k q", k=2)
                for j, (o, n) in enumerate(zip(OFFS, NVEC)):
                    nc.vector.tensor_reduce(
                        ZS3[:, :, j * r:(j + 1) * r], eu4[:, :, :, o:o + n],
                        axis=AX, op=add_op)
                Z = ZS[:, :6 * r]
                S = ZS[:, 6 * r:]
                rz = sm.tile([P, 6 * r], f32, tag="rz")
                nc.vector.reciprocal_approx_fast(rz[:], Z)
                L = sm.tile([P, 6 * r], f32, tag="L")
                for j in range(6):
                    nc.scalar.activation(
                        L[:, j * r:(j + 1) * r], Z[:, j * r:(j + 1) * r], Ln,
                        accum_out=accB[:, t * 6 + j: t * 6 + j + 1])
                to = sm.tile([P, 2 * r], f32, tag="to")
                for g, (j0, nj, n_seg) in enumerate(STT_GROUPS):
                    # to = (S_group * inv_n) * rz_group ; accA col = sum over group
                    nc.vector.scalar_tensor_tensor(
                        out=to[:, :nj * r],
                        in0=S[:, j0 * r:(j0 + nj) * r],
                        scalar=1.0 / n_seg,
                        in1=rz[:, j0 * r:(j0 + nj) * r],
                        op0=mult_op,
                        op1=mult_op,
                        accum_out=accA[:, t * 4 + g: t * 4 + g + 1])
            nc.sync.dma_start(acc_a[:], accA[:])
            nc.sync.dma_start(acc_b[:], accB[:])
    with _force_exp_ln_one_table_set():
        nc.compile()
    return nc, Tt


def _force_exp_ln_one_table_set():
    """Make the act-table pass map both Exp and Ln to
    natural_log_exp_and_others (otherwise it alternates exp_and_others /
    natural_log per tile: 14 ACT_TABLE_LOADs ~= 18us of ScalarE time).
    Set order (and thus act_func_set_id) is preserved; Exp/Ln are simply
    removed from every other set."""
    import contextlib
    import concourse.bacc as bacc_mod
    import concourse.mybir as mybir

    @contextlib.contextmanager
    def ctx():
        orig = bacc_mod.get_activation_tables

        def patched(arch):
            tables = {k: set(v) for k, v in orig(arch).items()}
            for name, funcs in tables.items():
                if name != "natural_log_exp_and_others":
                    funcs.discard(mybir.ActivationFunctionType.Exp)
                    funcs.discard(mybir.ActivationFunctionType.Ln)
            return tables

        bacc_mod.get_activation_tables = patched
        try:
            yield
        finally:
            bacc_mod.get_activation_tables = orig

    return ctx()


def _get_program():
    key = (W_CORE, R)
    if key not in _PROGRAM_CACHE:
        _PROGRAM_CACHE[key] = build_program(W_CORE, R)
    return _PROGRAM_CACHE[key]


def _host_x(current_action):
    """Segmented log_softmax of current_action in float64 on host."""
    ca = np.asarray(current_action, np.float64)
    x = np.empty(A, np.float64)
    for o, n in zip(OFFS, NVEC):
        seg = ca[o:o + n]
        m = seg.max()
        x[o:o + n] = seg - (m + np.log(np.exp(seg - m).sum()))
    return x


def combine_partials(results, w_full=W_FULL):
    """Combine per-core acc_a [P,T*4] (inv_n-weighted S/Z partials) and
    acc_b [P,T*6] (unweighted per-segment log-sums) into the scalar loss."""
    inv_n = 1.0 / np.asarray(NVEC, np.float64)
    total = 0.0
    for res in results:
        a = np.asarray(res["acc_a"], np.float64)
        b = np.asarray(res["acc_b"], np.float64)
        total += a.sum()  # inv_n already folded in on-device
        bt = b.reshape(P, -1, 6).sum(axis=(0, 1))  # [6] unweighted log-sums
        total -= (bt * inv_n).sum()
    return np.float32(total / w_full)


def kernel(current_action, previous_actions):
    from concourse import bass_utils

    nc, _ = _get_program()
    x = _host_x(current_action).astype(np.float32)
    xbt = np.broadcast_to(x, (P, A)).copy()
    pa = np.ascontiguousarray(np.asarray(previous_actions, np.float32))
    assert pa.shape == (W_FULL, A)
    in_maps = [
        {"pa": pa[c * W_CORE:(c + 1) * W_CORE], "xb": xbt}
        for c in range(N_CORES)
    ]
    res = bass_utils.run_bass_kernel_spmd(
        nc, in_maps, core_ids=list(range(N_CORES)))
    return combine_partials(res.results)


if __name__ == "__main__":
    np.random.seed(0)
    ca = np.random.randn(A).astype(np.float32)
    pa = np.random.randn(W_FULL, A).astype(np.float32)
    print(kernel(ca, pa))
